# revision 41
# baseline (speedup 1.0000x reference)
"""Trainium2 Bass kernel for nn_PolySelfAttention.

Reference computation (per batch b, head h, d = head dim):
    qkv = x @ w_qkv.T                     # [N, 3C] -> q,k,v [N, C] each
    mod[h,d]  = sum_n p2[h,n] * k[n,hd] * v[n,hd]
    modln     = layernorm_D(mod) * gamma + beta
    q'[n,hd]  = q[n,hd] * p1[h,n] * modln[h,d]
    out       = q' @ w_proj.T + b_proj

Sharding: data-parallel over B across the 8 cores (one batch per core, no
collectives).

Per-core kernel (natural [n, c] layout, bf16 matmuls / fp32 accumulate):
  pass 1: per 128-row n-tile: k,v = xT_tile.T @ w_kvT (PE); k evac on ACT;
          kvw2 = k*v*p2 (DVE, p2 broadcast via 0-stride AP); partition-
          reduce mod += ones.T @ kvw2 (PE) in 4-tile PSUM groups folded
          into an SBUF accumulator on DVE.
  LN over D on the [1, C] accumulator (DVE stats + ACT sqrt), broadcast
  across partitions with a ones[1,128] outer-product matmul, folded into
  the per-tile p1 expansion so pass-2 q matmuls never wait on the LN.
  pass 2: q = xT_tile.T @ w_qT (PE); q' = q * (p1*modln) (DVE);
          q'^T via PE transpose (bf16 identity); y = q'^T.T @ w_projT
          (PE) + bias (DVE) -> DMA out.

Engineering constraints encoded here (walrus/TRN2):
  - built as bacc.Bacc + nc.compile() so multi-sem waits are legalized
    (move_matmul_waits_to_ldweights / generate_event_semaphores); matmul,
    LW and DMA ISA structs accept only ONE sync wait each.
  - xpool bufs=8 matches the 8-lane SWDGE round-robin so tile-slot reuse
    pairs DMAs on the same lane (program-order WAW, no extra wait).
  - PSUM accumulators live only over small matmul groups; a long-lived
    PSUM accumulator makes Tile serialize same-bank matmuls.

Host-side prep (outside the measured device kernel): transposes of x and
weights + bf16 casts, p1/p2 transposes, bias replication.
"""

import os
import sys

for _p in ("/opt/trn_rl_repo", "/opt/pypackages"):
    if _p not in sys.path and os.path.isdir(_p):
        sys.path.append(_p)

import numpy as np
from contextlib import ExitStack

import concourse.bass as bass
import concourse.bacc as bacc
import concourse.tile as tile
from concourse import mybir, masks
from concourse._compat import with_exitstack
from concourse.bass_utils import run_bass_kernel_spmd

P = 128
C = 768
H = 12
D = 64
KT = C // P  # 6 contraction tiles
F32 = mybir.dt.float32
F32R = mybir.dt.float32r
BF16 = mybir.dt.bfloat16
CHUNK = 512
LN_EPS = 1e-5
N_CORES = 8


def _mm_chunks(total):
    return [(c0, min(CHUNK, total - c0)) for c0 in range(0, total, CHUNK)]


@with_exitstack
def poly_kernel(ctx: ExitStack, tc: tile.TileContext, n: int):
    nc = tc.nc
    nt = n // P
    mult = mybir.AluOpType.mult
    add = mybir.AluOpType.add
    subtract = mybir.AluOpType.subtract

    xT = nc.dram_tensor("xT", [C, n], BF16, kind="ExternalInput").ap()
    w_kvT = nc.dram_tensor("w_kvT", [C, 2 * C], BF16, kind="ExternalInput").ap()
    w_qT = nc.dram_tensor("w_qT", [C, C], BF16, kind="ExternalInput").ap()
    w_projT = nc.dram_tensor("w_projT", [C, C], BF16, kind="ExternalInput").ap()
    p1T = nc.dram_tensor("p1T", [n, H], F32, kind="ExternalInput").ap()
    p2T = nc.dram_tensor("p2T", [n, H], F32, kind="ExternalInput").ap()
    b_bc = nc.dram_tensor("b_bc", [P, C], F32, kind="ExternalInput").ap()
    gamma_t = nc.dram_tensor("gamma_t", [1, C], F32, kind="ExternalInput").ap()
    beta_t = nc.dram_tensor("beta_t", [1, C], F32, kind="ExternalInput").ap()
    out = nc.dram_tensor("out", [n, C], F32, kind="ExternalOutput").ap()

    # ---- persistent weights / constants ----
    wpool = ctx.enter_context(tc.tile_pool(name="weights", bufs=1))
    w_kv_sb = wpool.tile([P, KT * 2 * C], BF16)
    for k in range(KT):
        nc.sync.dma_start(
            w_kv_sb[:, k * 2 * C : (k + 1) * 2 * C],
            w_kvT[k * P : (k + 1) * P, :],
        )
    w_q_sb = wpool.tile([P, KT * C], BF16)
    for k in range(KT):
        nc.sync.dma_start(
            w_q_sb[:, k * C : (k + 1) * C], w_qT[k * P : (k + 1) * P, :]
        )
    w_pr_sb = wpool.tile([P, KT * C], BF16)
    for k in range(KT):
        nc.sync.dma_start(
            w_pr_sb[:, k * C : (k + 1) * C], w_projT[k * P : (k + 1) * P, :]
        )
    p1_sb = wpool.tile([P, nt * H], F32)
    nc.sync.dma_start(
        p1_sb[:].rearrange("p (t h) -> p t h", t=nt),
        p1T.rearrange("(t p) h -> p t h", p=P),
    )
    p2_sb = wpool.tile([P, nt * H], F32)
    nc.sync.dma_start(
        p2_sb[:].rearrange("p (t h) -> p t h", t=nt),
        p2T.rearrange("(t p) h -> p t h", p=P),
    )
    b_sb = wpool.tile([P, C], F32)
    nc.sync.dma_start(b_sb[:], b_bc[:])
    gamma_sb = wpool.tile([1, C], F32)
    nc.sync.dma_start(gamma_sb[:], gamma_t[:])
    beta_sb = wpool.tile([1, C], F32)
    nc.sync.dma_start(beta_sb[:], beta_t[:])
    ident = wpool.tile([P, P], BF16)
    ones_col = wpool.tile([P, 1], BF16)

    # bufs=8 matches the 8-lane SWDGE round-robin: slot reuse pairs each
    # DMA with the one 8 steps earlier on the SAME lane, so the WAW is
    # program-order and the DMA keeps a single sync wait (ISA limit 1).
    xpool = ctx.enter_context(tc.tile_pool(name="x", bufs=8))
    dvepool = ctx.enter_context(tc.tile_pool(name="dve", bufs=2))
    kvwpool = ctx.enter_context(tc.tile_pool(name="kvw", bufs=3))

    # ================= pass 1: k, v -> mod =================
    # The n-tile reduction accumulates in PSUM only over small groups of
    # tiles (fresh pooled tile per group), then folds into an SBUF
    # accumulator on DVE.  A single long-lived PSUM accumulator would make
    # Tile serialize every reduction matmul against the previous one with
    # same-bank completion fences, overflowing the 1-wait ISA slot on MM.
    GRP = 4
    mod_acc = wpool.tile([1, C], F32)
    n_grp = (nt + GRP - 1) // GRP
    with tc.tile_pool(name="kvps", bufs=2, space="PSUM") as kvpool, tc.tile_pool(
        name="modps", bufs=1, space="PSUM"
    ) as modpool:
        pending = None  # software-pipelined ones-matmul operand
        mod_part = None

        def emit_ones_mm(pi, pkvw):
            nonlocal mod_part
            g = pi // GRP
            first = pi % GRP == 0
            last = pi % GRP == GRP - 1 or pi == nt - 1
            if first:
                mod_part = modpool.tile([1, C], F32, tag="modpart")
            for c0, sz in _mm_chunks(C):
                nc.tensor.matmul(
                    mod_part[:, c0 : c0 + sz],
                    ones_col[:],
                    pkvw[:, c0 : c0 + sz],
                    start=first,
                    stop=last,
                )
            if last:
                if g == 0:
                    nc.vector.tensor_copy(mod_acc[:], mod_part[:])
                else:
                    nc.vector.tensor_tensor(
                        mod_acc[:], mod_acc[:], mod_part[:], op=add
                    )

        for i in range(nt):
            xt = xpool.tile([P, KT * P], BF16, tag="xt")
            nc.gpsimd.dma_start(
                xt[:].rearrange("p (k f) -> p k f", k=KT),
                xT.rearrange("(k p) n -> p k n", p=P)[:, :, i * P : (i + 1) * P],
            )
            if i == 0:
                # deferred so the Pool engine issues the first x-tile DMA
                # before any constant-building work
                nc.gpsimd.memset(ones_col[:], 1.0)
            if i == nt - 2:
                masks.make_identity(nc, ident[:])
            kv_ps = kvpool.tile([P, 2 * C], F32)
            for k in range(KT):
                lhs = xt[:, k * P : (k + 1) * P]
                for c0, sz in _mm_chunks(2 * C):
                    nc.tensor.matmul(
                        kv_ps[:, c0 : c0 + sz],
                        lhs,
                        w_kv_sb[:, k * 2 * C + c0 : k * 2 * C + c0 + sz],
                        start=(k == 0),
                        stop=(k == KT - 1),
                    )
            # emit previous tile's reduction matmuls now so PE never waits on DVE
            if pending is not None:
                emit_ones_mm(*pending)
            # DVE reads at most one PSUM operand per op: evacuate k first,
            # then k_sb * v(PSUM) on DVE.
            k_sb = dvepool.tile([P, C], F32, tag="ksb")
            nc.scalar.copy(k_sb[:], kv_ps[:, :C])
            kvw = dvepool.tile([P, C], F32, tag="kvw0")
            nc.vector.tensor_tensor(kvw[:], k_sb[:], kv_ps[:, C:], op=mult)
            p2e = dvepool.tile([P, C], F32, tag="p2e")
            nc.vector.tensor_copy(
                p2e[:].rearrange("p (h d) -> p h d", h=H),
                p2_sb[:, i * H : (i + 1) * H][:, :, None].broadcast_to([P, H, D]),
            )
            kvw2 = kvwpool.tile([P, C], BF16, tag="kvw2")
            nc.vector.tensor_tensor(kvw2[:], kvw[:], p2e[:], op=mult)
            pending = (i, kvw2)
        emit_ones_mm(*pending)

    # ================= layernorm on mod [1, C] =================
    lnpool = ctx.enter_context(tc.tile_pool(name="ln", bufs=1))
    mod_sb = mod_acc

    mod_hd = mod_sb[:].rearrange("p (h d) -> p h d", h=H)
    mean = lnpool.tile([1, H], F32)
    nc.vector.tensor_reduce(mean[:], mod_hd, axis=mybir.AxisListType.X, op=add)
    nc.vector.tensor_scalar_mul(mean[:], mean[:], 1.0 / D)
    sq = lnpool.tile([1, C], F32)
    nc.vector.tensor_tensor(sq[:], mod_sb[:], mod_sb[:], op=mult)
    sumsq = lnpool.tile([1, H], F32)
    nc.vector.tensor_reduce(
        sumsq[:], sq[:].rearrange("p (h d) -> p h d", h=H), axis=mybir.AxisListType.X, op=add
    )
    var = lnpool.tile([1, H], F32)
    nc.vector.tensor_scalar_mul(var[:], sumsq[:], 1.0 / D)
    meansq = lnpool.tile([1, H], F32)
    nc.vector.tensor_tensor(meansq[:], mean[:], mean[:], op=mult)
    nc.vector.tensor_tensor(var[:], var[:], meansq[:], op=subtract)
    eps_t = lnpool.tile([1, 1], F32)
    nc.gpsimd.memset(eps_t[:], LN_EPS)
    sstd = lnpool.tile([1, H], F32)
    nc.scalar.activation(
        sstd[:], var[:], mybir.ActivationFunctionType.Sqrt, bias=eps_t[:]
    )
    rstd = lnpool.tile([1, H], F32)
    nc.vector.reciprocal(rstd[:], sstd[:])
    modln = lnpool.tile([1, C], F32)
    modln_hd = modln[:].rearrange("p (h d) -> p h d", h=H)
    nc.vector.tensor_tensor(
        modln_hd, mod_hd, mean[:, :, None].broadcast_to([1, H, D]), op=subtract
    )
    nc.vector.tensor_tensor(
        modln_hd, modln_hd, rstd[:, :, None].broadcast_to([1, H, D]), op=mult
    )
    nc.vector.tensor_tensor(modln[:], modln[:], gamma_sb[:], op=mult)
    nc.vector.tensor_tensor(modln[:], modln[:], beta_sb[:], op=add)

    # broadcast modln across partitions (ones [1,P] outer product on PE, exact
    # fp32); modln is folded into the per-tile p1e build in pass 2, keeping
    # the q matmuls themselves independent of the layernorm.
    ones_row = lnpool.tile([1, P], F32)
    nc.gpsimd.memset(ones_row[:], 1.0)
    mb_sb = lnpool.tile([P, C], F32)

    with tc.tile_pool(name="mbps", bufs=1, space="PSUM") as mbpool:
        mb_ps = mbpool.tile([P, C], F32)
        for c0, sz in _mm_chunks(C):
            nc.tensor.matmul(
                mb_ps[:, c0 : c0 + sz],
                ones_row[:],
                modln[:, c0 : c0 + sz],
                start=True,
                stop=True,
            )
        nc.vector.tensor_copy(mb_sb[:], mb_ps[:])

    # ================= pass 2: q -> out =================
    with tc.tile_pool(name="qps", bufs=2, space="PSUM") as qpool, tc.tile_pool(
        name="trps", bufs=2, space="PSUM"
    ) as trpool, tc.tile_pool(name="yps", bufs=1, space="PSUM") as ypool:
        prev = None  # (qp tile, i) pipelined: transpose+proj of tile i-1 after q-mm of i

        def emit_tail(qp, i):
            qT = dvepool.tile([P, KT * P], BF16, tag="qT")
            for k in range(KT):
                tr_ps = trpool.tile([P, P], BF16, tag="trps")
                nc.tensor.transpose(tr_ps[:], qp[:, k * P : (k + 1) * P], ident[:])
                nc.vector.tensor_copy(qT[:, k * P : (k + 1) * P], tr_ps[:])
            y_ps = ypool.tile([P, C], F32, tag="yps")
            for k in range(KT):
                lhs = qT[:, k * P : (k + 1) * P]
                for c0, sz in _mm_chunks(C):
                    nc.tensor.matmul(
                        y_ps[:, c0 : c0 + sz],
                        lhs,
                        w_pr_sb[:, k * C + c0 : k * C + c0 + sz],
                        start=(k == 0),
                        stop=(k == KT - 1),
                    )
            y_sb = dvepool.tile([P, C], F32, tag="ysb")
            nc.vector.tensor_tensor(y_sb[:], y_ps[:], b_sb[:], op=add)
            nc.sync.dma_start(out[i * P : (i + 1) * P, :], y_sb[:])

        for i in range(nt):
            xt = xpool.tile([P, KT * P], BF16, tag="xt")
            nc.gpsimd.dma_start(
                xt[:].rearrange("p (k f) -> p k f", k=KT),
                xT.rearrange("(k p) n -> p k n", p=P)[:, :, i * P : (i + 1) * P],
            )
            q_ps = qpool.tile([P, C], F32, tag="qps")
            for k in range(KT):
                lhs = xt[:, k * P : (k + 1) * P]
                for c0, sz in _mm_chunks(C):
                    nc.tensor.matmul(
                        q_ps[:, c0 : c0 + sz],
                        lhs,
                        w_q_sb[:, k * C + c0 : k * C + c0 + sz],
                        start=(k == 0),
                        stop=(k == KT - 1),
                    )
            if prev is not None:
                emit_tail(*prev)
            p1e = dvepool.tile([P, C], F32, tag="p1e")
            nc.vector.tensor_tensor(
                p1e[:].rearrange("p (h d) -> p h d", h=H),
                p1_sb[:, i * H : (i + 1) * H][:, :, None].broadcast_to([P, H, D]),
                mb_sb[:].rearrange("p (h d) -> p h d", h=H),
                op=mult,
            )
            qp = kvwpool.tile([P, C], BF16, tag="qp")
            nc.vector.tensor_tensor(qp[:], q_ps[:], p1e[:], op=mult)
            prev = (qp, i)
        emit_tail(*prev)


_BUILD_CACHE = {}


def build_program(n):
    if n in _BUILD_CACHE:
        return _BUILD_CACHE[n]
    nc = bacc.Bacc("TRN2", target_bir_lowering=False, debug=False, num_devices=N_CORES)
    with tile.TileContext(nc) as tc:
        poly_kernel(tc, n)
    nc.compile()
    _BUILD_CACHE[n] = nc
    return nc


def host_prep(x, w_qkv, w_proj, b_proj, p1, p2, gamma, beta):
    """Build the per-core input maps (host-side layout prep)."""
    B = x.shape[0]
    import ml_dtypes

    bf16 = ml_dtypes.bfloat16
    w_kvT = np.ascontiguousarray(w_qkv[C : 3 * C].T.astype(bf16))
    w_qT = np.ascontiguousarray(w_qkv[:C].T.astype(bf16))
    w_projT = np.ascontiguousarray(w_proj.T.astype(bf16))
    p1T = np.ascontiguousarray(p1.T, dtype=np.float32)
    p2T = np.ascontiguousarray(p2.T, dtype=np.float32)
    b_bc = np.ascontiguousarray(np.broadcast_to(b_proj, (P, C)), dtype=np.float32)
    gamma_t = np.tile(np.asarray(gamma, np.float32), H)[None, :]
    beta_t = np.tile(np.asarray(beta, np.float32), H)[None, :]
    shared = dict(
        w_kvT=w_kvT, w_qT=w_qT, w_projT=w_projT, p1T=p1T, p2T=p2T,
        b_bc=b_bc, gamma_t=gamma_t, beta_t=beta_t,
    )
    return [
        {"xT": np.ascontiguousarray(x[b].T.astype(bf16)), **shared}
        for b in range(B)
    ]


def kernel(x, w_qkv, w_proj, b_proj, p1, p2, gamma, beta):
    x = np.asarray(x, dtype=np.float32)
    B, n, _ = x.shape
    nc = build_program(n)
    in_maps = host_prep(x, w_qkv, w_proj, b_proj, p1, p2, gamma, beta)
    res = run_bass_kernel_spmd(nc, in_maps, core_ids=list(range(B)))
    return np.stack([res.results[b]["out"] for b in range(B)], axis=0)
